# revision 39
# baseline (speedup 1.0000x reference)
"""CapsuleLayer dynamic-routing kernel for 8 Trainium2 NeuronCores.

Problem (hardcoded shapes):
  x: [B=64, R=2048, I=16] f32, W: [R=2048, C=16, O=32, I=16] f32
  u_hat[b,r,c,o] = sum_i W[r,c,o,i] * x[b,r,i]
  3 dynamic-routing iterations (softmax over c, squash over o) -> v [B, R, O]

Strategy:
  - Shard R across 8 cores (256 r's each). No collectives needed.
  - Host-side layout prep (not counted in HW time):
      * xblk[rp, 32, 128]: block-diag stationary for a pair of r's
        (K=(r_hat,i)=32, M=(r_hat,b)=128)
      * wm[rp, 32, 544]: moving operand: W[r,i,(c,o)] for the pair, plus 32
        extra columns holding mean_c W (folds iteration-0's uniform-softmax
        contraction into the same matmul).
  - Device: per r-pair chunk, PE computes u_hat [128=(r_hat,b), 512=(c,o)]
    and s0 [128, 32] in PSUM; routing runs on DVE/ACT/GPSIMD in fp32
    (bf16/tf32 break the routing: softmax logits ~ +-40 amplify errors).
"""

import numpy as np
import sys

sys.path.insert(0, "/opt/trn_rl_repo")

B, R, C, O, I = 64, 2048, 16, 32, 16
N_CORES = 8
R_SHARD = R // N_CORES          # 256
NPAIR = R_SHARD // 2            # 128 chunks per core
N_ITER = 3

_cache = {}


def _build_program(npair=NPAIR, reps=1, variant="dvemul_hwdma_g8_gps2"):
    """Build the Bass program once; returns nc. reps>1 repeats the whole
    computation (idempotent) for wall-clock-delta timing.
    variant: 'full' | 'nort' (no routing) | 'dvemul' (both big muls on DVE)
             | 'gpsmul' (both big muls on GPSIMD) | 'noalpha' (skip alpha
             chains, v_scale=const) | 'hwdma' (sync-engine DMA)."""
    from contextlib import ExitStack

    import concourse.bacc as bacc
    import concourse.tile as tile
    from concourse import mybir

    # The act-table-load pass assigns each activation the FIRST table set
    # containing its func: Copy/Exp/Square -> set 0, Ln -> set 5, causing a
    # ~2.7us table reload on nearly every activation. All four funcs coexist
    # in set "natural_log_exp_and_others"; blank out earlier sets (indices
    # must be preserved - they index the real act_info.json) so everything
    # lands on that one set => a single table load for the whole kernel.
    if not getattr(bacc, "_act_tables_patched", False):
        _orig_get_tables = bacc.get_activation_tables

        def _patched(arch):
            tabs = dict(_orig_get_tables(arch))
            target = "natural_log_exp_and_others"
            assert target in tabs
            return {
                name: (funcs if name == target else set())
                for name, funcs in tabs.items()
            }

        bacc.get_activation_tables = _patched
        bacc._act_tables_patched = True

    f32 = mybir.dt.float32
    AX = mybir.AxisListType
    ALU = mybir.AluOpType
    ACTF = mybir.ActivationFunctionType

    nc = bacc.Bacc("TRN2", target_bir_lowering=False, debug=False)

    # xw[rp, 32, 672]: [:, :, :128] = block-diag x stationary, [:, :, 128:672]
    # = W moving operand (512 u_hat cols + 32 mean_c-W cols). One DMA per
    # chunk => a single wait on each matmul (walrus sync-slot limit).
    xw = nc.dram_tensor("xw", [npair, 32, 672], f32, kind="ExternalInput")
    vout = nc.dram_tensor("vout", [B, 2 * npair, O], f32, kind="ExternalOutput")

    xw_ap = xw.ap()
    # view: [rp, r_hat, b, o] so a [128=(r_hat,b), 32] tile DMAs straight out
    vout_view = vout.ap().rearrange("b (rp two) o -> rp two b o", two=2)

    with tile.TileContext(nc) as tc, ExitStack() as ctx:
        xp = ctx.enter_context(tc.tile_pool(name="xp", bufs=12))
        psA = ctx.enter_context(tc.tile_pool(name="psA", bufs=4, space="PSUM"))
        psB = ctx.enter_context(tc.tile_pool(name="psB", bufs=4, space="PSUM"))
        up = ctx.enter_context(tc.tile_pool(name="up", bufs=10))
        tp = ctx.enter_context(tc.tile_pool(name="tp", bufs=8))
        sp = ctx.enter_context(tc.tile_pool(name="sp", bufs=14))
        sm = ctx.enter_context(tc.tile_pool(name="sm", bufs=4))

        dma_eng = nc.sync if "hwdma" in variant else nc.gpsimd
        G = 8 if "g8" in variant else 4  # chunks per phase-interleaved group
        while (npair * reps) % G:
            G //= 2

        def alpha_batch(squF, ZF, tagp):
            """Batched over a group: alpha*zi [128,G] from ||s_un||^2 and Z.
            alpha = sqrt(sig2)/(1+sig2), sig2 = squF*zi^2 (zi=1 if ZF None).
            sqrt via exp(0.5*ln(x)): keeps every ACT func in ONE table set
            (natural_log_exp_and_others) — a Sqrt op would force a ~2.7us
            ACT table reload on every Exp<->Sqrt alternation."""
            if ZF is not None:
                zi = sm.tile([128, G], f32, tag=tagp + "zi")
                nc.vector.reciprocal(zi, ZF)
                zi2 = sm.tile([128, G], f32, tag=tagp + "zi2")
                nc.vector.tensor_mul(zi2, zi, zi)
                sig2 = sm.tile([128, G], f32, tag=tagp + "sig2")
                nc.vector.tensor_mul(sig2, squF, zi2)
            else:
                sig2 = squF
            a1 = sm.tile([128, G], f32, tag=tagp + "a1")
            nc.vector.tensor_scalar_add(a1, sig2, 1.0)
            ra = sm.tile([128, G], f32, tag=tagp + "ra")
            nc.vector.reciprocal(ra, a1)
            lt = sm.tile([128, G], f32, tag=tagp + "lt")
            nc.scalar.activation(lt, sig2, ACTF.Ln)
            rt = sm.tile([128, G], f32, tag=tagp + "rt")
            nc.scalar.activation(rt, lt, ACTF.Exp, scale=0.5)
            al = sm.tile([128, G], f32, tag=tagp + "al")
            nc.vector.tensor_mul(al, rt, ra)
            if ZF is not None:
                az = sm.tile([128, G], f32, tag=tagp + "az")
                nc.vector.tensor_mul(az, al, zi)
                return az
            return al

        rps = [i for _ in range(reps) for i in range(npair)]
        assert len(rps) % G == 0
        for g0 in range(0, len(rps), G):
            grp = rps[g0:g0 + G]

            # P0/P1: loads + matmuls
            us, ss = [], []
            for rp in grp:
                xwt = xp.tile([32, 672], f32)
                dma_eng.dma_start(out=xwt, in_=xw_ap[rp])
                u_ps = psA.tile([128, 512], f32)
                nc.tensor.matmul(u_ps, lhsT=xwt[:, :128], rhs=xwt[:, 128:640],
                                 start=True, stop=True)
                s0_ps = psB.tile([128, O], f32)
                nc.tensor.matmul(s0_ps, lhsT=xwt[:, :128], rhs=xwt[:, 640:],
                                 start=True, stop=True)
                us.append((u_ps, s0_ps))

            # P2: evacuate + ||s0||^2 into group state
            u_sb, s_cur = [], []
            squ0 = sm.tile([128, G], f32, tag="squ0")
            for j, (u_ps, s0_ps) in enumerate(us):
                u = up.tile([128, 512], f32)
                nc.scalar.copy(u, u_ps)      # ACT evacuates PSUM
                u_sb.append(u.rearrange("p (c o) -> p c o", o=O))
                junk = sp.tile([128, O], f32, tag="junk")
                if "s0psum" in variant:
                    # read s0 straight from PSUM (ACT PSUM-src is cheaper
                    # than SBUF-src); it-1's broadcast mul also reads PSUM
                    s_cur.append(s0_ps)
                    nc.scalar.activation(junk, s0_ps, ACTF.Square,
                                         accum_out=squ0[:, j:j + 1])
                else:
                    s = sp.tile([128, O], f32, tag="s0")
                    nc.scalar.copy(s, s0_ps)
                    s_cur.append(s)
                    nc.scalar.activation(junk, s, ACTF.Square,
                                         accum_out=squ0[:, j:j + 1])
            # P3: batched alpha0 (Z=1: c uniform via mean_c-W matmul columns)
            vsc = alpha_batch(squ0, None, "a0")

            b_cur = [None] * G
            for it in (1, 2):
                # P4/P6: per-chunk agreement + softmax + s_unnorm
                squF = sm.tile([128, G], f32, tag=f"squ{it}")
                ZF = sm.tile([128, G], f32, tag=f"Z{it}")
                mF = sm.tile([128, G], f32, tag=f"m{it}")
                nmF = sm.tile([128, G], f32, tag=f"nm{it}")
                s_next = []
                for j in range(G):
                    u3 = u_sb[j]
                    t1 = tp.tile([128, 16, O], f32, tag="t1")
                    s_b = s_cur[j].unsqueeze(1).broadcast_to((128, 16, O))
                    if ("gps2" in variant) or ("gps1" in variant and it == 2):
                        nc.gpsimd.tensor_tensor(t1, u3, s_b, op=ALU.mult)
                    else:
                        nc.vector.tensor_mul(t1, u3, s_b)
                    bd = sp.tile([128, 16], f32, tag="bd")
                    nc.vector.reduce_sum(bd, t1, axis=AX.X)
                    b_new = sp.tile([128, 16], f32, tag="bnew")
                    if b_cur[j] is None:
                        nc.vector.tensor_scalar_mul(b_new, bd, vsc[:, j:j + 1])
                    else:
                        nc.vector.scalar_tensor_tensor(
                            out=b_new, in0=bd, scalar=vsc[:, j:j + 1],
                            in1=b_cur[j], op0=ALU.mult, op1=ALU.add)
                    b_cur[j] = b_new
                    nc.vector.reduce_max(mF[:, j:j + 1], b_new, axis=AX.X)
                for j in range(G):
                    # nm batched would serialize; per-slice negate is one op
                    nc.vector.tensor_scalar_mul(nmF[:, j:j + 1], mF[:, j:j + 1], -1.0)
                    e = sp.tile([128, 16], f32, tag="e")
                    nc.scalar.activation(e, b_cur[j], ACTF.Exp,
                                         bias=nmF[:, j:j + 1], scale=1.0,
                                         accum_out=ZF[:, j:j + 1])
                    t2 = tp.tile([128, 16, O], f32, tag="t2")
                    e_b = e.unsqueeze(2).broadcast_to((128, 16, O))
                    nc.vector.tensor_mul(t2, u_sb[j], e_b)
                    s = sp.tile([128, O], f32, tag="s")
                    nc.vector.reduce_sum(s, t2.transpose([0, 2, 1]), axis=AX.X)
                    s_next.append(s)
                    junk = sp.tile([128, O], f32, tag="junk")
                    nc.scalar.activation(junk, s, ACTF.Square,
                                         accum_out=squF[:, j:j + 1])
                # P5/P7: batched alpha chain
                vsc = alpha_batch(squF, ZF, f"a{it}")
                s_cur = s_next

            # P8: scale + store
            for j, rp in enumerate(grp):
                vt = sp.tile([128, O], f32, tag="vt")
                nc.scalar.mul(vt, s_cur[j], mul=vsc[:, j:j + 1])
                dma_eng.dma_start(out=vout_view[rp], in_=vt)

    nc.compile()
    return nc


def _prep_inputs(x, W):
    """Host-side sharding + layout prep. Returns list of in_maps per core."""
    x = np.ascontiguousarray(x, dtype=np.float32)
    W = np.ascontiguousarray(W, dtype=np.float32)
    in_maps = []
    for k in range(N_CORES):
        r0 = k * R_SHARD
        xs = x[:, r0:r0 + R_SHARD, :]              # [B, 256, I]
        Ws = W[r0:r0 + R_SHARD]                    # [256, C, O, I]

        xw = np.zeros((NPAIR, 32, 672), np.float32)
        # block-diag x stationary: rows (r_hat*16+i), cols (r_hat*64+b)
        xT = xs.transpose(1, 2, 0)                 # [256, I, B]
        xw[:, :16, :64] = xT[0::2]
        xw[:, 16:, 64:128] = xT[1::2]
        # W moving: [:, r_hat*16+i, 128 + c*32+o] = W[r, c, o, i]
        Wt = Ws.transpose(0, 3, 1, 2).reshape(R_SHARD, I, C * O)   # [256, I, 512]
        xw[:, :16, 128:640] = Wt[0::2]
        xw[:, 16:, 128:640] = Wt[1::2]
        wbar = Wt.reshape(R_SHARD, I, C, O).mean(axis=2)           # [256, I, O]
        xw[:, :16, 640:] = wbar[0::2]
        xw[:, 16:, 640:] = wbar[1::2]

        in_maps.append({"xw": xw})
    return in_maps


def kernel(x, W, _trace=False):
    from concourse import bass_utils

    if "nc" not in _cache:
        _cache["nc"] = _build_program()
    nc = _cache["nc"]

    in_maps = _prep_inputs(x, W)
    res = bass_utils.run_bass_kernel_spmd(
        nc, in_maps, core_ids=list(range(N_CORES)), trace=_trace)
    _cache["last_result"] = res

    out = np.empty((B, R, O), np.float32)
    for k in range(N_CORES):
        out[:, k * R_SHARD:(k + 1) * R_SHARD, :] = res.results[k]["vout"]
    return out


# revision 41
# speedup vs baseline: 1.0499x; 1.0499x over previous
"""CapsuleLayer dynamic-routing kernel for 8 Trainium2 NeuronCores.

Problem (hardcoded shapes):
  x: [B=64, R=2048, I=16] f32, W: [R=2048, C=16, O=32, I=16] f32
  u_hat[b,r,c,o] = sum_i W[r,c,o,i] * x[b,r,i]
  3 dynamic-routing iterations (softmax over c, squash over o) -> v [B, R, O]

Strategy:
  - Shard R across 8 cores (256 r's each). No collectives needed.
  - Host-side layout prep (not counted in HW time):
      * xblk[rp, 32, 128]: block-diag stationary for a pair of r's
        (K=(r_hat,i)=32, M=(r_hat,b)=128)
      * wm[rp, 32, 544]: moving operand: W[r,i,(c,o)] for the pair, plus 32
        extra columns holding mean_c W (folds iteration-0's uniform-softmax
        contraction into the same matmul).
  - Device: per r-pair chunk, PE computes u_hat [128=(r_hat,b), 512=(c,o)]
    and s0 [128, 32] in PSUM; routing runs on DVE/ACT/GPSIMD in fp32
    (bf16/tf32 break the routing: softmax logits ~ +-40 amplify errors).
"""

import numpy as np
import sys

sys.path.insert(0, "/opt/trn_rl_repo")

B, R, C, O, I = 64, 2048, 16, 32, 16
N_CORES = 8
R_SHARD = R // N_CORES          # 256
NPAIR = R_SHARD // 2            # 128 chunks per core
N_ITER = 3

_cache = {}


def _build_program(npair=NPAIR, reps=1, variant="dvemul_hwdma_g8_gps2"):
    """Build the Bass program once; returns nc. reps>1 repeats the whole
    computation (idempotent) for wall-clock-delta timing.
    variant: 'full' | 'nort' (no routing) | 'dvemul' (both big muls on DVE)
             | 'gpsmul' (both big muls on GPSIMD) | 'noalpha' (skip alpha
             chains, v_scale=const) | 'hwdma' (sync-engine DMA)."""
    from contextlib import ExitStack

    import concourse.bacc as bacc
    import concourse.tile as tile
    from concourse import mybir

    # The act-table-load pass assigns each activation the FIRST table set
    # containing its func: Copy/Exp/Square -> set 0, Ln -> set 5, causing a
    # ~2.7us table reload on nearly every activation. All four funcs coexist
    # in set "natural_log_exp_and_others"; blank out earlier sets (indices
    # must be preserved - they index the real act_info.json) so everything
    # lands on that one set => a single table load for the whole kernel.
    if not getattr(bacc, "_act_tables_patched", False):
        _orig_get_tables = bacc.get_activation_tables

        def _patched(arch):
            tabs = dict(_orig_get_tables(arch))
            target = "natural_log_exp_and_others"
            assert target in tabs
            return {
                name: (funcs if name == target else set())
                for name, funcs in tabs.items()
            }

        bacc.get_activation_tables = _patched
        bacc._act_tables_patched = True

    f32 = mybir.dt.float32
    AX = mybir.AxisListType
    ALU = mybir.AluOpType
    ACTF = mybir.ActivationFunctionType

    nc = bacc.Bacc("TRN2", target_bir_lowering=False, debug=False)

    # xw[rp, 32, 672]: [:, :, :128] = block-diag x stationary, [:, :, 128:672]
    # = W moving operand (512 u_hat cols + 32 mean_c-W cols). One DMA per
    # chunk => a single wait on each matmul (walrus sync-slot limit).
    xw = nc.dram_tensor("xw", [npair, 32, 672], f32, kind="ExternalInput")
    vout = nc.dram_tensor("vout", [B, 2 * npair, O], f32, kind="ExternalOutput")

    xw_ap = xw.ap()
    # view: [rp, r_hat, b, o] so a [128=(r_hat,b), 32] tile DMAs straight out
    vout_view = vout.ap().rearrange("b (rp two) o -> rp two b o", two=2)

    with tile.TileContext(nc) as tc, ExitStack() as ctx:
        xp = ctx.enter_context(tc.tile_pool(name="xp", bufs=12))
        psA = ctx.enter_context(tc.tile_pool(name="psA", bufs=4, space="PSUM"))
        psB = ctx.enter_context(tc.tile_pool(name="psB", bufs=4, space="PSUM"))
        up = ctx.enter_context(tc.tile_pool(name="up", bufs=10))
        tp = ctx.enter_context(tc.tile_pool(name="tp", bufs=8))
        sp = ctx.enter_context(tc.tile_pool(name="sp", bufs=14))
        sm = ctx.enter_context(tc.tile_pool(name="sm", bufs=4))

        dma_eng = nc.sync if "hwdma" in variant else nc.gpsimd
        G = 8 if "g8" in variant else 4  # chunks per phase-interleaved group
        while (npair * reps) % G:
            G //= 2

        def alpha_batch(squF, ZF, tagp):
            """Batched over a group: alpha*zi [128,G] from ||s_un||^2 and Z.
            alpha = sqrt(sig2)/(1+sig2), sig2 = squF*zi^2 (zi=1 if ZF None).
            sqrt via exp(0.5*ln(x)): keeps every ACT func in ONE table set
            (natural_log_exp_and_others) — a Sqrt op would force a ~2.7us
            ACT table reload on every Exp<->Sqrt alternation."""
            if ZF is not None:
                zi = sm.tile([128, G], f32, tag=tagp + "zi")
                nc.vector.reciprocal(zi, ZF)
                zi2 = sm.tile([128, G], f32, tag=tagp + "zi2")
                nc.vector.tensor_mul(zi2, zi, zi)
                sig2 = sm.tile([128, G], f32, tag=tagp + "sig2")
                nc.vector.tensor_mul(sig2, squF, zi2)
            else:
                sig2 = squF
            a1 = sm.tile([128, G], f32, tag=tagp + "a1")
            nc.vector.tensor_scalar_add(a1, sig2, 1.0)
            ra = sm.tile([128, G], f32, tag=tagp + "ra")
            nc.vector.reciprocal(ra, a1)
            lt = sm.tile([128, G], f32, tag=tagp + "lt")
            nc.scalar.activation(lt, sig2, ACTF.Ln)
            rt = sm.tile([128, G], f32, tag=tagp + "rt")
            nc.scalar.activation(rt, lt, ACTF.Exp, scale=0.5)
            al = sm.tile([128, G], f32, tag=tagp + "al")
            nc.vector.tensor_mul(al, rt, ra)
            if ZF is not None:
                az = sm.tile([128, G], f32, tag=tagp + "az")
                nc.vector.tensor_mul(az, al, zi)
                return az
            return al

        rps = [i for _ in range(reps) for i in range(npair)]
        assert len(rps) % G == 0
        for g0 in range(0, len(rps), G):
            grp = rps[g0:g0 + G]

            # P0/P1: loads + matmuls
            us, ss = [], []
            for rp in grp:
                xwt = xp.tile([32, 672], f32)
                dma_eng.dma_start(out=xwt, in_=xw_ap[rp])
                u_ps = psA.tile([128, 512], f32)
                nc.tensor.matmul(u_ps, lhsT=xwt[:, :128], rhs=xwt[:, 128:640],
                                 start=True, stop=True)
                s0_ps = psB.tile([128, O], f32)
                nc.tensor.matmul(s0_ps, lhsT=xwt[:, :128], rhs=xwt[:, 640:],
                                 start=True, stop=True)
                us.append((u_ps, s0_ps))

            # P2: evacuate + ||s0||^2 into group state
            u_sb, s_cur = [], []
            squ0 = sm.tile([128, G], f32, tag="squ0")
            for j, (u_ps, s0_ps) in enumerate(us):
                u = up.tile([128, 512], f32)
                nc.scalar.copy(u, u_ps)      # ACT evacuates PSUM
                u_sb.append(u.rearrange("p (c o) -> p c o", o=O))
                junk = sp.tile([128, O], f32, tag="junk")
                if "s0psum" in variant:
                    # read s0 straight from PSUM (ACT PSUM-src is cheaper
                    # than SBUF-src); it-1's broadcast mul also reads PSUM
                    s_cur.append(s0_ps)
                    nc.scalar.activation(junk, s0_ps, ACTF.Square,
                                         accum_out=squ0[:, j:j + 1])
                else:
                    s = sp.tile([128, O], f32, tag="s0")
                    nc.scalar.copy(s, s0_ps)
                    s_cur.append(s)
                    nc.scalar.activation(junk, s, ACTF.Square,
                                         accum_out=squ0[:, j:j + 1])
            # P3: batched alpha0 (Z=1: c uniform via mean_c-W matmul columns)
            vsc = alpha_batch(squ0, None, "a0")

            b_cur = [None] * G
            for it in (1, 2):
                # P4/P6: per-chunk agreement + softmax + s_unnorm
                squF = sm.tile([128, G], f32, tag=f"squ{it}")
                ZF = sm.tile([128, G], f32, tag=f"Z{it}")
                mF = sm.tile([128, G], f32, tag=f"m{it}")
                nmF = sm.tile([128, G], f32, tag=f"nm{it}")
                s_next = []
                for j in range(G):
                    u3 = u_sb[j]
                    t1 = tp.tile([128, 16, O], f32, tag="t1")
                    s_b = s_cur[j].unsqueeze(1).broadcast_to((128, 16, O))
                    if ("gps2" in variant) or ("gps1" in variant and it == 2):
                        nc.gpsimd.tensor_tensor(t1, u3, s_b, op=ALU.mult)
                    else:
                        nc.vector.tensor_mul(t1, u3, s_b)
                    bd = sp.tile([128, 16], f32, tag="bd")
                    nc.vector.reduce_sum(bd, t1, axis=AX.X)
                    b_new = sp.tile([128, 16], f32, tag="bnew")
                    if b_cur[j] is None:
                        nc.vector.tensor_scalar_mul(b_new, bd, vsc[:, j:j + 1])
                    else:
                        nc.vector.scalar_tensor_tensor(
                            out=b_new, in0=bd, scalar=vsc[:, j:j + 1],
                            in1=b_cur[j], op0=ALU.mult, op1=ALU.add)
                    b_cur[j] = b_new
                    # negate=True: out = -max, directly usable as the exp bias
                    nc.vector.reduce_max(nmF[:, j:j + 1], b_new, axis=AX.X,
                                         negate=True)
                for j in range(G):
                    e = sp.tile([128, 16], f32, tag="e")
                    nc.scalar.activation(e, b_cur[j], ACTF.Exp,
                                         bias=nmF[:, j:j + 1], scale=1.0,
                                         accum_out=ZF[:, j:j + 1])
                    t2 = tp.tile([128, 16, O], f32, tag="t2")
                    e_b = e.unsqueeze(2).broadcast_to((128, 16, O))
                    if "gps3" in variant and it == 2:
                        nc.gpsimd.tensor_tensor(t2, u_sb[j], e_b, op=ALU.mult)
                    else:
                        nc.vector.tensor_mul(t2, u_sb[j], e_b)
                    s = sp.tile([128, O], f32, tag="s")
                    nc.vector.reduce_sum(s, t2.transpose([0, 2, 1]), axis=AX.X)
                    s_next.append(s)
                    junk = sp.tile([128, O], f32, tag="junk")
                    nc.scalar.activation(junk, s, ACTF.Square,
                                         accum_out=squF[:, j:j + 1])
                # P5/P7: batched alpha chain
                vsc = alpha_batch(squF, ZF, f"a{it}")
                s_cur = s_next

            # P8: scale + store
            for j, rp in enumerate(grp):
                vt = sp.tile([128, O], f32, tag="vt")
                nc.scalar.mul(vt, s_cur[j], mul=vsc[:, j:j + 1])
                dma_eng.dma_start(out=vout_view[rp], in_=vt)

    nc.compile()
    return nc


def _prep_inputs(x, W):
    """Host-side sharding + layout prep. Returns list of in_maps per core."""
    x = np.ascontiguousarray(x, dtype=np.float32)
    W = np.ascontiguousarray(W, dtype=np.float32)
    in_maps = []
    for k in range(N_CORES):
        r0 = k * R_SHARD
        xs = x[:, r0:r0 + R_SHARD, :]              # [B, 256, I]
        Ws = W[r0:r0 + R_SHARD]                    # [256, C, O, I]

        xw = np.zeros((NPAIR, 32, 672), np.float32)
        # block-diag x stationary: rows (r_hat*16+i), cols (r_hat*64+b)
        xT = xs.transpose(1, 2, 0)                 # [256, I, B]
        xw[:, :16, :64] = xT[0::2]
        xw[:, 16:, 64:128] = xT[1::2]
        # W moving: [:, r_hat*16+i, 128 + c*32+o] = W[r, c, o, i]
        Wt = Ws.transpose(0, 3, 1, 2).reshape(R_SHARD, I, C * O)   # [256, I, 512]
        xw[:, :16, 128:640] = Wt[0::2]
        xw[:, 16:, 128:640] = Wt[1::2]
        wbar = Wt.reshape(R_SHARD, I, C, O).mean(axis=2)           # [256, I, O]
        xw[:, :16, 640:] = wbar[0::2]
        xw[:, 16:, 640:] = wbar[1::2]

        in_maps.append({"xw": xw})
    return in_maps


def kernel(x, W, _trace=False):
    from concourse import bass_utils

    if "nc" not in _cache:
        _cache["nc"] = _build_program()
    nc = _cache["nc"]

    in_maps = _prep_inputs(x, W)
    res = bass_utils.run_bass_kernel_spmd(
        nc, in_maps, core_ids=list(range(N_CORES)), trace=_trace)
    _cache["last_result"] = res

    out = np.empty((B, R, O), np.float32)
    for k in range(N_CORES):
        out[:, k * R_SHARD:(k + 1) * R_SHARD, :] = res.results[k]["vout"]
    return out


# revision 45
# speedup vs baseline: 1.4508x; 1.3818x over previous
"""CapsuleLayer dynamic-routing kernel for 8 Trainium2 NeuronCores.

Problem (hardcoded shapes):
  x: [B=64, R=2048, I=16] f32, W: [R=2048, C=16, O=32, I=16] f32
  u_hat[b,r,c,o] = sum_i W[r,c,o,i] * x[b,r,i]
  3 dynamic-routing iterations (softmax over c, squash over o) -> v [B, R, O]

Strategy:
  - Shard R across 8 cores (256 r's each). No collectives needed.
  - Host-side layout prep (not counted in HW time):
      * xblk[rp, 32, 128]: block-diag stationary for a pair of r's
        (K=(r_hat,i)=32, M=(r_hat,b)=128)
      * wm[rp, 32, 544]: moving operand: W[r,i,(c,o)] for the pair, plus 32
        extra columns holding mean_c W (folds iteration-0's uniform-softmax
        contraction into the same matmul).
  - Device: per r-pair chunk, PE computes u_hat [128=(r_hat,b), 512=(c,o)]
    and s0 [128, 32] in PSUM; routing runs on DVE/ACT/GPSIMD in fp32
    (bf16/tf32 break the routing: softmax logits ~ +-40 amplify errors).
"""

import numpy as np
import sys

sys.path.insert(0, "/opt/trn_rl_repo")

B, R, C, O, I = 64, 2048, 16, 32, 16
N_CORES = 8
R_SHARD = R // N_CORES          # 256
NPAIR = R_SHARD // 2            # 128 chunks per core
N_ITER = 3

_cache = {}


def _build_program(npair=NPAIR, reps=1, variant="dvemul_hwdma_g8_gps2"):
    """Build the Bass program once; returns nc. reps>1 repeats the whole
    computation (idempotent) for wall-clock-delta timing.
    variant: 'full' | 'nort' (no routing) | 'dvemul' (both big muls on DVE)
             | 'gpsmul' (both big muls on GPSIMD) | 'noalpha' (skip alpha
             chains, v_scale=const) | 'hwdma' (sync-engine DMA)."""
    from contextlib import ExitStack

    import concourse.bacc as bacc
    import concourse.tile as tile
    from concourse import mybir

    # The act-table-load pass assigns each activation the FIRST table set
    # containing its func: Copy/Exp/Square -> set 0, Ln -> set 5, causing a
    # ~2.7us table reload on nearly every activation. All four funcs coexist
    # in set "natural_log_exp_and_others"; blank out earlier sets (indices
    # must be preserved - they index the real act_info.json) so everything
    # lands on that one set => a single table load for the whole kernel.
    if not getattr(bacc, "_act_tables_patched", False):
        _orig_get_tables = bacc.get_activation_tables

        def _patched(arch):
            tabs = dict(_orig_get_tables(arch))
            target = "natural_log_exp_and_others"
            assert target in tabs
            return {
                name: (funcs if name == target else set())
                for name, funcs in tabs.items()
            }

        bacc.get_activation_tables = _patched
        bacc._act_tables_patched = True

    f32 = mybir.dt.float32
    AX = mybir.AxisListType
    ALU = mybir.AluOpType
    ACTF = mybir.ActivationFunctionType

    nc = bacc.Bacc("TRN2", target_bir_lowering=False, debug=False)

    # xw[rp, 32, 672]: [:, :, :128] = block-diag x stationary, [:, :, 128:672]
    # = W moving operand (512 u_hat cols + 32 mean_c-W cols). One DMA per
    # chunk => a single wait on each matmul (walrus sync-slot limit).
    xw = nc.dram_tensor("xw", [npair, 32, 672], f32, kind="ExternalInput")
    vout = nc.dram_tensor("vout", [B, 2 * npair, O], f32, kind="ExternalOutput")

    xw_ap = xw.ap()
    # view: [rp, r_hat, b, o] so a [128=(r_hat,b), 32] tile DMAs straight out
    vout_view = vout.ap().rearrange("b (rp two) o -> rp two b o", two=2)

    with tile.TileContext(nc) as tc, ExitStack() as ctx:
        xp = ctx.enter_context(tc.tile_pool(name="xp", bufs=16))
        nA = 6 if "psa6" in variant else 4
        psA = ctx.enter_context(tc.tile_pool(name="psA", bufs=nA, space="PSUM"))
        psB = ctx.enter_context(tc.tile_pool(name="psB", bufs=8 - nA, space="PSUM"))
        up = ctx.enter_context(tc.tile_pool(name="up", bufs=14))
        tp = ctx.enter_context(tc.tile_pool(name="tp", bufs=12))
        sp = ctx.enter_context(tc.tile_pool(name="sp", bufs=14))
        sm = ctx.enter_context(tc.tile_pool(name="sm", bufs=4))

        dma_eng = nc.sync if "hwdma" in variant else nc.gpsimd
        G = 16 if "g16" in variant else (8 if "g8" in variant else 4)
        while (npair * reps) % G:
            G //= 2

        def alpha_batch(squF, ZF, tagp):
            """Batched over a group: alpha*zi [128,G] from ||s_un||^2 and Z.
            alpha = sqrt(sig2)/(1+sig2), sig2 = squF*zi^2 (zi=1 if ZF None).
            sqrt via exp(0.5*ln(x)): keeps every ACT func in ONE table set
            (natural_log_exp_and_others) — a Sqrt op would force a ~2.7us
            ACT table reload on every Exp<->Sqrt alternation."""
            if ZF is not None:
                zi = sm.tile([128, G], f32, tag=tagp + "zi")
                nc.vector.reciprocal(zi, ZF)
                zi2 = sm.tile([128, G], f32, tag=tagp + "zi2")
                nc.vector.tensor_mul(zi2, zi, zi)
                sig2 = sm.tile([128, G], f32, tag=tagp + "sig2")
                nc.vector.tensor_mul(sig2, squF, zi2)
            else:
                sig2 = squF
            a1 = sm.tile([128, G], f32, tag=tagp + "a1")
            nc.vector.tensor_scalar_add(a1, sig2, 1.0)
            ra = sm.tile([128, G], f32, tag=tagp + "ra")
            nc.vector.reciprocal(ra, a1)
            lt = sm.tile([128, G], f32, tag=tagp + "lt")
            nc.scalar.activation(lt, sig2, ACTF.Ln)
            rt = sm.tile([128, G], f32, tag=tagp + "rt")
            nc.scalar.activation(rt, lt, ACTF.Exp, scale=0.5)
            al = sm.tile([128, G], f32, tag=tagp + "al")
            nc.vector.tensor_mul(al, rt, ra)
            if ZF is not None:
                az = sm.tile([128, G], f32, tag=tagp + "az")
                nc.vector.tensor_mul(az, al, zi)
                return az
            return al

        rps = [i for _ in range(reps) for i in range(npair)]
        assert len(rps) % G == 0
        for g0 in range(0, len(rps), G):
            grp = rps[g0:g0 + G]

            # P0/P1: loads + matmuls
            us, ss = [], []
            for rp in grp:
                xwt = xp.tile([32, 672], f32)
                dma_eng.dma_start(out=xwt, in_=xw_ap[rp])
                u_ps = psA.tile([128, 512], f32)
                nc.tensor.matmul(u_ps, lhsT=xwt[:, :128], rhs=xwt[:, 128:640],
                                 start=True, stop=True)
                s0_ps = psB.tile([128, O], f32)
                nc.tensor.matmul(s0_ps, lhsT=xwt[:, :128], rhs=xwt[:, 640:],
                                 start=True, stop=True)
                us.append((u_ps, s0_ps))

            # P2: evacuate + ||s0||^2 into group state
            u_sb, s_cur = [], []
            squ0 = sm.tile([128, G], f32, tag="squ0")
            for j, (u_ps, s0_ps) in enumerate(us):
                u = up.tile([128, 512], f32)
                nc.scalar.copy(u, u_ps)      # ACT evacuates PSUM
                u_sb.append(u.rearrange("p (c o) -> p c o", o=O))
                junk = sp.tile([128, O], f32, tag="junk")
                if "s0psum" in variant:
                    # read s0 straight from PSUM (ACT PSUM-src is cheaper
                    # than SBUF-src); it-1's broadcast mul also reads PSUM
                    s_cur.append(s0_ps)
                    nc.scalar.activation(junk, s0_ps, ACTF.Square,
                                         accum_out=squ0[:, j:j + 1])
                else:
                    s = sp.tile([128, O], f32, tag="s0")
                    nc.scalar.copy(s, s0_ps)
                    s_cur.append(s)
                    nc.scalar.activation(junk, s, ACTF.Square,
                                         accum_out=squ0[:, j:j + 1])
            # P3: batched alpha0 (Z=1: c uniform via mean_c-W matmul columns)
            vsc = alpha_batch(squ0, None, "a0")

            b_cur = [None] * G
            for it in (1, 2):
                # P4/P6: per-chunk agreement + softmax + s_unnorm
                squF = sm.tile([128, G], f32, tag=f"squ{it}")
                ZF = sm.tile([128, G], f32, tag=f"Z{it}")
                mF = sm.tile([128, G], f32, tag=f"m{it}")
                nmF = sm.tile([128, G], f32, tag=f"nm{it}")
                s_next = []
                for j in range(G):
                    u3 = u_sb[j]
                    t1 = tp.tile([128, 16, O], f32, tag="t1")
                    s_b = s_cur[j].unsqueeze(1).broadcast_to((128, 16, O))
                    if ("gps2" in variant) or ("gps1" in variant and it == 2):
                        nc.gpsimd.tensor_tensor(t1, u3, s_b, op=ALU.mult)
                    else:
                        nc.vector.tensor_mul(t1, u3, s_b)
                    bd = sp.tile([128, 16], f32, tag="bd")
                    nc.vector.reduce_sum(bd, t1, axis=AX.X)
                    b_new = sp.tile([128, 16], f32, tag="bnew")
                    if b_cur[j] is None:
                        nc.vector.tensor_scalar_mul(b_new, bd, vsc[:, j:j + 1])
                    else:
                        nc.vector.scalar_tensor_tensor(
                            out=b_new, in0=bd, scalar=vsc[:, j:j + 1],
                            in1=b_cur[j], op0=ALU.mult, op1=ALU.add)
                    b_cur[j] = b_new
                    # negate=True: out = -max, directly usable as the exp bias
                    nc.vector.reduce_max(nmF[:, j:j + 1], b_new, axis=AX.X,
                                         negate=True)
                for j in range(G):
                    e = sp.tile([128, 16], f32, tag="e")
                    nc.scalar.activation(e, b_cur[j], ACTF.Exp,
                                         bias=nmF[:, j:j + 1], scale=1.0,
                                         accum_out=ZF[:, j:j + 1])
                    t2 = tp.tile([128, 16, O], f32, tag="t2")
                    e_b = e.unsqueeze(2).broadcast_to((128, 16, O))
                    if "gps3" in variant and it == 2:
                        nc.gpsimd.tensor_tensor(t2, u_sb[j], e_b, op=ALU.mult)
                    else:
                        nc.vector.tensor_mul(t2, u_sb[j], e_b)
                    s = sp.tile([128, O], f32, tag="s")
                    nc.vector.reduce_sum(s, t2.transpose([0, 2, 1]), axis=AX.X)
                    s_next.append(s)
                    junk = sp.tile([128, O], f32, tag="junk")
                    nc.scalar.activation(junk, s, ACTF.Square,
                                         accum_out=squF[:, j:j + 1])
                # P5/P7: batched alpha chain
                vsc = alpha_batch(squF, ZF, f"a{it}")
                s_cur = s_next

            # P8: scale + store
            for j, rp in enumerate(grp):
                vt = sp.tile([128, O], f32, tag="vt")
                nc.scalar.mul(vt, s_cur[j], mul=vsc[:, j:j + 1])
                dma_eng.dma_start(out=vout_view[rp], in_=vt)

    nc.compile()
    return nc


def _prep_inputs(x, W):
    """Host-side sharding + layout prep. Returns list of in_maps per core."""
    x = np.ascontiguousarray(x, dtype=np.float32)
    W = np.ascontiguousarray(W, dtype=np.float32)
    in_maps = []
    for k in range(N_CORES):
        r0 = k * R_SHARD
        xs = x[:, r0:r0 + R_SHARD, :]              # [B, 256, I]
        Ws = W[r0:r0 + R_SHARD]                    # [256, C, O, I]

        xw = np.zeros((NPAIR, 32, 672), np.float32)
        # block-diag x stationary: rows (r_hat*16+i), cols (r_hat*64+b)
        xT = xs.transpose(1, 2, 0)                 # [256, I, B]
        xw[:, :16, :64] = xT[0::2]
        xw[:, 16:, 64:128] = xT[1::2]
        # W moving: [:, r_hat*16+i, 128 + c*32+o] = W[r, c, o, i]
        Wt = Ws.transpose(0, 3, 1, 2).reshape(R_SHARD, I, C * O)   # [256, I, 512]
        xw[:, :16, 128:640] = Wt[0::2]
        xw[:, 16:, 128:640] = Wt[1::2]
        wbar = Wt.reshape(R_SHARD, I, C, O).mean(axis=2)           # [256, I, O]
        xw[:, :16, 640:] = wbar[0::2]
        xw[:, 16:, 640:] = wbar[1::2]

        in_maps.append({"xw": xw})
    return in_maps


def kernel(x, W, _trace=False):
    from concourse import bass_utils

    if "nc" not in _cache:
        _cache["nc"] = _build_program()
    nc = _cache["nc"]

    in_maps = _prep_inputs(x, W)
    res = bass_utils.run_bass_kernel_spmd(
        nc, in_maps, core_ids=list(range(N_CORES)), trace=_trace)
    _cache["last_result"] = res

    out = np.empty((B, R, O), np.float32)
    for k in range(N_CORES):
        out[:, k * R_SHARD:(k + 1) * R_SHARD, :] = res.results[k]["vout"]
    return out
